# revision 56
# baseline (speedup 1.0000x reference)
"""Trainium2 Bass kernel for GQA decode attention (nn_Attention_37890201485423).

Tensor-parallel over KV heads: 8 cores x (1 KV head + 5 query heads each).
o_proj is row-sharded; the cross-core reduction is a single ReduceScatter
(input [8 batches, 5120] row-major -> core i ends with batch i's summed
output row; the host stacks the 8 rows, a pure unshard).

Layout strategy (all host-side prep is numpy; all module math runs on device):
  - All large arrays ship to HBM as bf16 (the matmuls consume bf16 operands;
    the f32 baseline cast during the DMA anyway). This halves HBM traffic,
    the binding resource: ~32.5 MB/core at ~330 GB/s effective = ~100 us.
  - K cache shard is shipped pre-transposed per batch as [d=128, t=4096] so
    the scores matmul uses K^T chunks as the stationary operand directly.
  - V cache shard is shipped as 128-token chunks with a ones column appended
    ([t, 129]); the AV matmul then produces numerator and softmax denominator
    in one accumulation group.
  - RoPE runs on device as one matmul with a 128x128 block-diagonal rotation
    matrix built from freqs_cis (f32 for precision; everything else bf16 with
    fp32 PSUM accumulation). Measured rel err ~6e-3 (gate 2e-2).

Schedule (the DMA stream is the critical resource; everything chases it):
  - One ordered SWDGE stream: wqkv -> whole per-batch K/V transfers (8KB
    lines; 6-deep tile prefetch so buffer recycling never stalls the stream
    while attention catches up) -> wo in 10 chunks.
  - A completion barrier keys wo to K7's completion: order-only deps would
    let wo bytes interleave with late KV batches across the 16 queues,
    delaying V7 and serializing o_proj behind the whole stream; anchoring on
    K7 lets wo's descriptor-gen overlap only V7's transfer. The six o_proj
    rounds (4x1024 + 2x512 cols, PSUM double-buffered per size) chase the wo
    chunk arrivals and finish ~3 us after the last byte.
  - Software pipelining: scores(b+1) is emitted before AV(b) so the PE fills
    the ACT exp latency; attention starts ~1.5 batches behind the stream
    (projections+rope gate it) and catches up by mid-stream.
  - Heater matmuls (dep-free junk on the PE) pad the per-batch stop-start
    bursts into sustained activity; without them the HAM clock gate never
    sees a fully-busy 3.4us window and every matmul runs at 1.2 GHz instead
    of 2.4 (cold attention falls BEHIND the DMA stream).
  - o_proj bias is folded into the DVE PSUM->SBUF cast (tensor_add with a
    host-replicated [8, 5120] bias), removing all bias matmuls.
  - A tiny warmup ReduceScatter, pinned early into the gpsimd chain, absorbs
    the ncfw cold start (unpinned, the Tile scheduler parks it right before
    the real RS where it serializes with it).
  - Constants (identity, memsets) are issued at the head of the gpsimd queue:
    behind the big dma_starts they run 20+ us late (ring backpressure) and
    stall the first projection matmul.

Measured: ~147-162 us on 8 cores (median ~153; baseline was ~245; rare
skew outliers to ~225). Core 0's span includes cross-core launch skew
(5-30 us, run-to-run) absorbed at the final ReduceScatter, plus ~7 us RS
mesh stages and ~6 us end-of-program drain. The local pipeline (first
instruction -> RS trigger) is ~116-119 us against a ~108 us DMA-stream
floor (32.5 MB/core at ~330 GB/s effective + preamble).
"""

import sys

import numpy as np

if "/opt/trn_rl_repo" not in sys.path:
    sys.path.insert(0, "/opt/trn_rl_repo")


def _install_ntff_hook():
    """The container's antenv stub lacks axon_hooks; recreate it so
    run_bass_kernel_spmd(trace=True) can capture NTFF profiles via the
    libaxon ctypes path (mirrors trn_agent_boot.trn_boot)."""
    import types

    if "antenv.axon_hooks" in sys.modules:
        return
    mod = types.ModuleType("antenv.axon_hooks")
    mod._hook = None

    def set_axon_ntff_profile_hook(h):
        mod._hook = h

    def get_axon_ntff_profile_hook():
        return mod._hook

    mod.set_axon_ntff_profile_hook = set_axon_ntff_profile_hook
    mod.get_axon_ntff_profile_hook = get_axon_ntff_profile_hook
    sys.modules["antenv.axon_hooks"] = mod
    try:
        import antenv

        antenv.axon_hooks = mod
    except ImportError:
        pass
    try:
        boot_dir = "/root/.axon_site/trn_agent_boot"
        if boot_dir not in sys.path:
            sys.path.insert(0, boot_dir)
        import trn_boot

        hook = trn_boot._ntff_profile_via_ctypes("/opt/axon/libaxon_pjrt.so")
        if hook is not None:
            mod._hook = hook
    except Exception:
        pass


_install_ntff_hook()

DIM, N_HEADS, N_KV, HEAD_DIM = 5120, 40, 8, 128
MAX_BS, MAX_SEQ = 8, 4096
NB = 8  # batch
N_CORES = 8
N_REP = N_HEADS // N_KV  # 5 query heads per kv head
HPC = N_REP  # heads per core
QD = HPC * HEAD_DIM  # 640, per-core q/o width
WKV = QD + 2 * HEAD_DIM  # 896: concat q|k|v projection width per core
KD = 40  # contraction chunks for DIM
NCH = 32  # 4096 / 128 token chunks
VE = HEAD_DIM + 1  # 129, v chunk width with ones column
# o_proj rounds (out col base, width): 4x1024 + 2x512 cols, double-buffered
# in PSUM per size-tag, each round chasing its wo chunk's DMA arrival (wo is
# last in the DMA stream); the last round is small to shorten the tail chain.
OP_ROUNDS = [(0, 1024), (1024, 1024), (2048, 1024), (3072, 1024), (4096, 512), (4608, 512)]
SCALE = 1.0 / float(np.sqrt(HEAD_DIM))
WARMUP_RS = True


def _build_rope_matrix(freqs_cis: np.ndarray) -> np.ndarray:
    """lhsT for the rope matmul: out = lhsT.T @ rhs applies the rotation A.

    A[2i,2i]=cos_i, A[2i,2i+1]=-sin_i, A[2i+1,2i]=sin_i, A[2i+1,2i+1]=cos_i
    (matches reference _apply_rope with interleaved even/odd pairs).
    """
    cos = np.asarray(freqs_cis, np.float32)[0, :, 0]
    sin = np.asarray(freqs_cis, np.float32)[0, :, 1]
    A = np.zeros((HEAD_DIM, HEAD_DIM), np.float32)
    idx = np.arange(HEAD_DIM // 2)
    A[2 * idx, 2 * idx] = cos
    A[2 * idx, 2 * idx + 1] = -sin
    A[2 * idx + 1, 2 * idx] = sin
    A[2 * idx + 1, 2 * idx + 1] = cos
    return np.ascontiguousarray(A.T)


def _part_major(w: np.ndarray) -> np.ndarray:
    """[K*128, N] -> [128, K*N] with chunk k in columns k*N:(k+1)*N."""
    k = w.shape[0] // 128
    return np.ascontiguousarray(
        w.reshape(k, 128, w.shape[1]).transpose(1, 0, 2).reshape(128, -1)
    )


def prepare_inputs(x, freqs_cis, cache_k, cache_v, wq, bq, wk, bk, wv, bv, wo, bo):
    """Returns per-core in_maps (list of dicts of numpy arrays).

    All large arrays are shipped to HBM as bf16 (the matmuls consume bf16
    operands anyway — the f32 baseline cast during the DMA; pre-casting on
    the host halves HBM traffic, which is the bottleneck). arope stays f32
    so the rope rotation keeps f32 precision.
    """
    import ml_dtypes

    bf16 = ml_dtypes.bfloat16
    x = np.asarray(x, np.float32).reshape(NB, DIM)
    arope = _build_rope_matrix(freqs_cis)

    # xt: stationary operand layout for the projections.
    # xt[:, k*8:(k+1)*8][p, c] = x[c, k*128+p]
    xs = x.reshape(NB, KD, 128)
    xt = np.ascontiguousarray(xs.transpose(2, 1, 0).reshape(128, KD * NB))

    # replicated across the 8 batch rows so the DVE can add it during the
    # o_proj PSUM->SBUF cast (kills the 10 bias matmuls on the PE)
    bo8 = np.tile((np.asarray(bo, np.float32) / N_CORES).reshape(1, DIM), (NB, 1))
    wq, wk, wv = (np.asarray(a, np.float32) for a in (wq, wk, wv))
    bqf = np.asarray(bq, np.float32).reshape(N_HEADS * HEAD_DIM)
    bkf = np.asarray(bk, np.float32).reshape(N_KV * HEAD_DIM)
    bvf = np.asarray(bv, np.float32).reshape(N_KV * HEAD_DIM)

    in_maps = []
    for i in range(N_CORES):
        # concat q|k|v slices: [5120, 896]
        w_cat = np.concatenate(
            [
                wq[:, i * QD : (i + 1) * QD],
                wk[:, i * HEAD_DIM : (i + 1) * HEAD_DIM],
                wv[:, i * HEAD_DIM : (i + 1) * HEAD_DIM],
            ],
            axis=1,
        )
        b_cat = np.concatenate(
            [
                bqf[i * QD : (i + 1) * QD],
                bkf[i * HEAD_DIM : (i + 1) * HEAD_DIM],
                bvf[i * HEAD_DIM : (i + 1) * HEAD_DIM],
            ]
        ).reshape(1, WKV)
        # wo: partition-major, then column-round-major so each o_proj round's
        # block is one contiguous DMA chunk
        wo_pm3 = _part_major(
            np.asarray(wo, np.float32)[i * QD : (i + 1) * QD, :]
        ).reshape(128, HPC, DIM)
        wo_i = np.ascontiguousarray(
            np.concatenate(
                [
                    wo_pm3[:, :, cb : cb + cw].reshape(128, HPC * cw)
                    for cb, cw in OP_ROUNDS
                ],
                axis=1,
            )
        )
        kt_i = np.asarray(cache_k, np.float32)[:, :, i, :].transpose(
            0, 2, 1
        )  # [8, 128, 4096]
        v_raw = np.asarray(cache_v, np.float32)[:, :, i, :].reshape(
            NB, NCH, 128, HEAD_DIM
        )
        v_ext = np.ones((NB, NCH, 128, VE), np.float32)
        v_ext[..., :HEAD_DIM] = v_raw
        v_i = v_ext.transpose(0, 2, 1, 3).reshape(NB, 128, NCH * VE)  # [8,128,4128]
        kv_i = np.ascontiguousarray(
            np.concatenate([kt_i, v_i], axis=2)
        )  # [8, 128, 8224]
        in_maps.append(
            dict(
                xt=xt.astype(bf16),
                wqkv=_part_major(w_cat).astype(bf16),
                bqkv=np.ascontiguousarray(b_cat).astype(bf16),
                arope=arope,
                kv=kv_i.astype(bf16),
                wo=wo_i.astype(bf16),
                bo8=bo8.astype(bf16),
            )
        )
    return in_maps


def build_graph():
    import concourse.mybir as mybir
    from concourse import bacc
    from concourse.masks import make_identity
    from concourse.tile import TileContext

    from concourse.bass import _add_dep_helper

    f32 = mybir.dt.float32
    bf16 = mybir.dt.bfloat16
    nc = bacc.Bacc(num_devices=N_CORES, name="attn_decode_tp8")

    # Pin the big-DMA stream order (the Tile scheduler otherwise reorders the
    # gpsimd queue and can push weight loads behind the KV cache stream).
    _prev_dma = [None]

    def ordered(bi):
        if _prev_dma[0] is not None:
            _add_dep_helper(
                bi.ins, _prev_dma[0].ins, sync=False, reason="dma stream order"
            )
        _prev_dma[0] = bi
        return bi

    xt_p = nc.declare_dram_parameter("xt", [128, KD * NB], bf16, isOutput=False)
    wqkv_p = nc.declare_dram_parameter("wqkv", [128, KD * WKV], bf16, isOutput=False)
    bqkv_p = nc.declare_dram_parameter("bqkv", [1, WKV], bf16, isOutput=False)
    arope_p = nc.declare_dram_parameter("arope", [128, 128], f32, isOutput=False)
    kv_p = nc.declare_dram_parameter(
        "kv", [NB, 128, MAX_SEQ + NCH * VE], bf16, isOutput=False
    )
    wo_p = nc.declare_dram_parameter("wo", [128, HPC * DIM], bf16, isOutput=False)
    bo8_p = nc.declare_dram_parameter("bo8", [NB, DIM], bf16, isOutput=False)
    # per-core output: batch-row core_id of the full [NB, DIM] result
    # (ReduceScatter leaves core i with row i; the host stacks the rows)
    out_p = nc.declare_dram_parameter("out", [1, DIM], f32, isOutput=True)

    Exp = mybir.ActivationFunctionType.Exp

    with TileContext(nc, num_cores=N_CORES) as tc:
        with (
            tc.tile_pool(name="const", bufs=1) as constp,
            tc.tile_pool(name="persist", bufs=1) as pers,
            tc.tile_pool(name="dram", bufs=1, space="DRAM") as dramp,
            tc.tile_pool(name="ktp", bufs=3) as ktp,
        ):
            identity = constp.tile([128, 128], f32)
            ones8 = constp.tile([1, NB], bf16)
            bo8_sb = constp.tile([NB, DIM], bf16)
            arope_sb = constp.tile([128, 128], f32)
            # const compute at the very head of the gpsimd queue: the big
            # dma_starts below backpressure the gpsimd engine on the DMA ring,
            # so anything queued after them runs 20+ us late — and the first
            # projection matmul needs ones8
            make_identity(nc, identity)
            nc.gpsimd.memset(ones8[:], 1.0)
            nc.sync.dma_start(arope_sb[:], arope_p[:])
            bqkv_sb = constp.tile([1, WKV], bf16)
            ordered(nc.gpsimd.dma_start(bqkv_sb[:], bqkv_p[:]))

            # tiny warm-up collective: absorbs the ncfw cold-start cost so the
            # real output ReduceScatter at the tail runs at steady-state latency
            warm_in = dramp.tile([1, NB * N_CORES], bf16)
            warm_out = dramp.tile([1, NB], bf16)
            nc.sync.dma_start(warm_in[:], bqkv_p[:, 0 : NB * N_CORES])
            # pin the warmup trigger into the ordered gpsimd chain — left to
            # the Tile scheduler it lands right before the real ReduceScatter
            # and serializes with it instead of warming the ncfw path early
            if WARMUP_RS:
                ordered(
                    nc.gpsimd.collective_compute(
                        "ReduceScatter",
                        mybir.AluOpType.add,
                        replica_groups=[list(range(N_CORES))],
                        ins=[warm_in.opt()],
                        outs=[warm_out.opt()],
                    )
                )

            qr_sb = pers.tile([128, NB * HPC], bf16)  # roped q^T, cols b*5+h
            knT_sb = pers.tile([128, NB], bf16)  # roped new-k^T, cols b
            xv_sb = pers.tile([NB, HEAD_DIM], bf16)  # new v rows
            # flattened copy on partition 0 (matmul operands need base
            # partition 0): cols b*VE..b*VE+127 = v_new[b], col b*VE+128 = 1.0
            xvf_sb = pers.tile([1, NB * VE], bf16)
            nc.gpsimd.memset(xvf_sb[:], 1.0)
            attnT_sb = pers.tile([128, HPC * NB], bf16)  # cols h*8+b

            kt_tiles, v_tiles = {}, {}
            kv_last = [None]  # last kv DMA instruction
            kv_barrier = [None]  # V7 first half: wo gen overlaps only the last ~1MB
            KVW = MAX_SEQ + NCH * VE  # 8224
            HCH = NCH // 2  # chunks per exp half

            def load_kv(bp):
                # pair-mode: 2 batches per transfer (K-pair then V-pair,
                # ~2MB each, 8 transfers total) — per-transfer ring-handoff
                # dead time was costing the 16-transfer variant ~6-8us of
                # kv-phase wall. bufs=4 holds all 8 batches: no recycling.
                kvb = ktp.tile(
                    [128, 2 * KVW], bf16, tag="kvb", bufs=4, name=f"kvb{bp}"
                )
                dst3 = kvb[:].rearrange("p (b w) -> p b w", b=2)
                src3 = kv_p[2 * bp : 2 * bp + 2].rearrange("b p w -> p b w")
                kv_barrier[0] = ordered(
                    nc.gpsimd.dma_start(
                        dst3[:, :, 0:MAX_SEQ], src3[:, :, 0:MAX_SEQ]
                    )
                )
                kv_last[0] = ordered(
                    nc.gpsimd.dma_start(
                        dst3[:, :, MAX_SEQ:KVW], src3[:, :, MAX_SEQ:KVW]
                    )
                )
                for o in range(2):
                    b = 2 * bp + o
                    kt_tiles[b] = kvb[:, o * KVW : o * KVW + MAX_SEQ]
                    v_tiles[b] = kvb[:, o * KVW + MAX_SEQ : (o + 1) * KVW]

            # ---------------- Phase A: projections + rope ----------------
            with (
                tc.tile_pool(name="pha", bufs=1) as pha,
                tc.tile_pool(name="wqp", bufs=2) as wqp,
                tc.tile_pool(name="phaps", bufs=1, space="PSUM") as phaps,
            ):
                xt_sb = pha.tile([128, KD * NB], bf16)
                ordered(nc.gpsimd.dma_start(xt_sb[:], xt_p[:]))

                GK = 10  # k-chunks per wqkv tile (4 tiles: smaller SBUF stage, earlier start)
                wq_tiles = []
                wq_last = None
                for g in range(KD // GK):
                    wq_sb = wqp.tile(
                        [128, GK * WKV], bf16, tag="wq", name=f"wq{g}"
                    )
                    wq_last = ordered(
                        nc.gpsimd.dma_start(
                            wq_sb[:], wqkv_p[:, g * GK * WKV : (g + 1) * GK * WKV]
                        )
                    )
                    wq_tiles.append(wq_sb)
                # start the first KV cache loads right behind the weights
                load_kv(0)
                load_kv(1)
                load_kv(2)
                # bo8 is only needed in Phase C — keep it off the queue head
                ordered(nc.gpsimd.dma_start(bo8_sb[:], bo8_p[:]))

                qkv_ps = phaps.tile([NB, WKV], f32)
                nc.tensor.matmul(
                    qkv_ps[:, 0:512],
                    ones8[:],
                    bqkv_sb[:, 0:512],
                    start=True,
                    stop=False,
                )
                nc.tensor.matmul(
                    qkv_ps[:, 512:WKV],
                    ones8[:],
                    bqkv_sb[:, 512:WKV],
                    start=True,
                    stop=False,
                )
                for g in range(KD // GK):
                    wq_sb = wq_tiles[g]
                    for o in range(GK):
                        k = g * GK + o
                        lhsT = xt_sb[:, k * NB : (k + 1) * NB]
                        last = k == KD - 1
                        nc.tensor.matmul(
                            qkv_ps[:, 0:512],
                            lhsT,
                            wq_sb[:, o * WKV : o * WKV + 512],
                            start=False,
                            stop=last,
                        )
                        nc.tensor.matmul(
                            qkv_ps[:, 512:WKV],
                            lhsT,
                            wq_sb[:, o * WKV + 512 : (o + 1) * WKV],
                            start=False,
                            stop=last,
                        )

                # copy q|k parts to f32 for transposes; v part to bf16
                qk_sb = pha.tile([NB, QD + HEAD_DIM], f32)
                nc.vector.tensor_copy(qk_sb[:], qkv_ps[:, 0 : QD + HEAD_DIM])
                nc.vector.tensor_copy(xv_sb[:], qkv_ps[:, QD + HEAD_DIM : WKV])
                nc.sync.dma_start(
                    xvf_sb.rearrange("p (b e) -> p b e", e=VE)[:, :, 0:HEAD_DIM],
                    xv_sb[:],
                )

                qkT_sb = pha.tile([128, NB * HPC + NB], f32)
                qkT_q3 = qkT_sb[:, 0 : NB * HPC].rearrange("p (b h) -> p b h", h=HPC)
                for h in range(HPC):
                    tq_ps = phaps.tile([128, NB], f32, tag="tq", bufs=2)
                    nc.tensor.transpose(
                        tq_ps[:],
                        qk_sb[:, h * HEAD_DIM : (h + 1) * HEAD_DIM],
                        identity[0:NB, 0:NB],
                    )
                    nc.vector.tensor_copy(qkT_q3[:, :, h], tq_ps[:])
                tk_ps = phaps.tile([128, NB], f32, tag="tq", bufs=2)
                nc.tensor.transpose(
                    tk_ps[:], qk_sb[:, QD : QD + HEAD_DIM], identity[0:NB, 0:NB]
                )
                nc.vector.tensor_copy(qkT_sb[:, NB * HPC : NB * HPC + NB], tk_ps[:])

                qkr_ps = phaps.tile([128, NB * HPC + NB], f32)
                nc.tensor.matmul(
                    qkr_ps[:], arope_sb[:], qkT_sb[:], start=True, stop=True
                )
                nc.vector.tensor_copy(qr_sb[:], qkr_ps[:, 0 : NB * HPC])
                nc.vector.tensor_copy(
                    knT_sb[:], qkr_ps[:, NB * HPC : NB * HPC + NB]
                )

            # ---------------- Phase B: attention per batch ----------------
            # Straight per-batch order (scores -> exp -> AV -> normalize ->
            # transpose); per-batch K/V arrivals keep the PE chasing the DMA
            # stream with only sub-us ACT/DVE ping-pong stalls.
            wo_tiles = []
            wop_cm = tc.tile_pool(name="wop", bufs=1)
            wop = wop_cm.__enter__()
            with (
                tc.tile_pool(name="psm", bufs=2) as psm,
                tc.tile_pool(name="aps", bufs=2, space="PSUM") as aps,
            ):
                attnT_3 = attnT_sb.rearrange("p (h b) -> p h b", b=NB)

                s_tiles, sn_tiles = {}, {}

                def scores(b):
                    ktb = kt_tiles[b]
                    s_ps = aps.tile([128, NCH * HPC], f32, tag="s", name=f"s{b}")
                    for c in range(NCH):
                        # last chunk: only 127 rows — cache position 4095 is
                        # stale (the new token is handled separately below)
                        w = 127 if c == NCH - 1 else 128
                        nc.tensor.matmul(
                            s_ps[0:w, c * HPC : (c + 1) * HPC],
                            ktb[:, c * 128 : c * 128 + w],
                            qr_sb[:, b * HPC : (b + 1) * HPC],
                            start=True,
                            stop=True,
                        )
                    # new token's score row: [1, 5] from its roped k column
                    sn_ps = aps.tile([1, HPC], f32, tag="sn", name=f"sn{b}")
                    nc.tensor.matmul(
                        sn_ps[:],
                        knT_sb[:, b : b + 1],
                        qr_sb[:, b * HPC : (b + 1) * HPC],
                        start=True,
                        stop=True,
                    )
                    s_tiles[b], sn_tiles[b] = s_ps, sn_ps

                scores(0)
                for b in range(NB):
                    if b == 0:
                        load_kv(3)  # batches 6-7
                    # software pipeline: scores(b+1) is emitted before
                    # AV(b) so the PE fills the ACT exp(b) latency and the
                    # per-batch serial PE->ACT->PE chain overlaps across
                    # batches (catch-up rate ~4.3us/batch instead of ~6)
                    if b + 1 < NB:
                        scores(b + 1)
                    # ---- AV + normalize for batch b ----
                    s_ps, sn_ps = s_tiles[b], sn_tiles[b]
                    vb = v_tiles[b]
                    p_sb = psm.tile([128, NCH * HPC], bf16, tag="p")
                    pn_sb = psm.tile([1, HPC], bf16, tag="pn")
                    for hf in range(2):
                        sl = slice(hf * HCH * HPC, (hf + 1) * HCH * HPC)
                        nc.scalar.activation(
                            p_sb[:, sl], s_ps[:, sl], Exp, scale=SCALE
                        )
                    nc.scalar.activation(pn_sb[:], sn_ps[:], Exp, scale=SCALE)
                    o_ps = aps.tile([HPC, VE], f32, tag="o")
                    for c in range(NCH):
                        w = 127 if c == NCH - 1 else 128
                        nc.tensor.matmul(
                            o_ps[:],
                            p_sb[0:w, c * HPC : (c + 1) * HPC],
                            vb[0:w, c * VE : (c + 1) * VE],
                            start=(c == 0),
                            stop=False,
                        )
                    # new token's AV contribution (k=1 contraction)
                    nc.tensor.matmul(
                        o_ps[:],
                        pn_sb[:],
                        xvf_sb[:, b * VE : (b + 1) * VE],
                        start=False,
                        stop=True,
                    )
                    r_sb = psm.tile([HPC, 1], f32, tag="r")
                    nc.vector.reciprocal(r_sb[:], o_ps[:, HEAD_DIM : HEAD_DIM + 1])
                    attn_b = psm.tile([HPC, HEAD_DIM], f32, tag="attn_b")
                    nc.vector.tensor_scalar_mul(
                        attn_b[:], o_ps[:, 0:HEAD_DIM], r_sb[:]
                    )
                    ta_ps = aps.tile([128, HPC], f32, tag="ta", bufs=1)
                    nc.tensor.transpose(
                        ta_ps[:], attn_b[:], identity[0:HPC, 0:HPC]
                    )
                    nc.vector.tensor_copy(attnT_3[:, :, b], ta_ps[:])
                    # heater matmuls: fill the per-batch DMA-wait gap with
                    # dep-free junk work so the PE's HAM activity window stays
                    # saturated — otherwise the stop-start attention bursts
                    # never sustain the ~3.4us needed to hold the 2.4 GHz
                    # clock and every real matmul runs at half speed
                    # Heater budget: early batches (attention catching up to
                    # the stream after the projections+rope head start) get a
                    # full fill; late batches are data-gated with real idle,
                    # so a few heaters keep HAM warm into the o_proj rounds;
                    # batch 7 gets none (it's on the tail critical path).
                    nheat = 3 if b % 2 == 0 else 6
                    if nheat:
                        heat_ps = aps.tile([128, 128], f32, tag="heat", bufs=1)
                        for _ in range(nheat):
                            nc.tensor.matmul(
                                heat_ps[:], arope_sb[:], arope_sb[:],
                                start=True, stop=True,
                            )

                # wo stream: strictly after the last KV byte (a completion
                # barrier — order-only deps would let wo bytes interleave
                # with late KV batches and delay V7, serializing o_proj
                # behind the whole stream), in 10 chunks on the
                # gpsimd chain so the o_proj rounds chase the arrivals
                wo_sb = wop.tile([128, HPC * DIM], bf16)
                W = HPC * DIM
                NWC = 10
                for q in range(NWC):
                    bi = ordered(
                        nc.gpsimd.dma_start(
                            wo_sb[:, q * W // NWC : (q + 1) * W // NWC],
                            wo_p[:, q * W // NWC : (q + 1) * W // NWC],
                        )
                    )
                    if q == 0:
                        # key the barrier to V7's FIRST half: wo descriptor
                        # gen + first bytes overlap only the last ~1MB of KV,
                        # cutting the post-stream bubble without starving the
                        # attention tail
                        _add_dep_helper(
                            bi.ins,
                            kv_barrier[0].ins,
                            sync=True,
                            reason="wo after kv stream (V7 1st half)",
                        )
                wo_tiles.append(wo_sb)

            # ---------------- Phase C: o_proj + single AllReduce ----------------
            # 5 rounds of 1024 out-cols, double-buffered in PSUM; each round's
            # DVE cast + staging DMA overlaps the next round's matmuls. One
            # AllReduce over the full [8, 5120] bf16 output at the end (one
            # ~8us mesh traversal instead of two serialized ones).
            with (
                tc.tile_pool(name="opsp", bufs=1, space="PSUM") as opsp,
                tc.tile_pool(name="oop", bufs=1) as oop,
            ):
                # single ReduceScatter at the end: the last collective's entry
                # is bounded by the slowest core's chain end no matter how the
                # RS is phased, so extra phases just append their ~7us mesh.
                cc_in = dramp.tile([NB, DIM], bf16, tag="cci")
                cc_out = dramp.tile([1, DIM], bf16, tag="cco")

                for r, (cb, cw) in enumerate(OP_ROUNDS):
                    blk = HPC * cb  # wo tile base of this round's block
                    op_ps = opsp.tile(
                        [NB, cw], f32, tag=f"op{cw}", bufs=2, name=f"op{r}"
                    )
                    for h in range(HPC):
                        for j in range(cw // 512):
                            nc.tensor.matmul(
                                op_ps[:, j * 512 : (j + 1) * 512],
                                attnT_sb[:, h * NB : (h + 1) * NB],
                                wo_tiles[0][
                                    :,
                                    blk + h * cw + j * 512 : blk + h * cw + (j + 1) * 512,
                                ],
                                start=(h == 0),
                                stop=(h == HPC - 1),
                            )
                    oo_sb = oop.tile(
                        [NB, cw], bf16, tag=f"oo{cw}", bufs=2, name=f"oo{r}"
                    )
                    # cast + bias in one DVE pass
                    nc.vector.tensor_add(
                        oo_sb[:], op_ps[:], bo8_sb[:, cb : cb + cw]
                    )
                    nc.sync.dma_start(cc_in[:, cb : cb + cw], oo_sb[:])
                # ReduceScatter semantics: input [NB, DIM] row-major -> chunk
                # i = batch-row i, so core i ends up with batch i's summed
                # output; the host stacks the rows (pure unshard). Half the
                # wire of an AllReduce.
                nc.gpsimd.collective_compute(
                    "ReduceScatter",
                    mybir.AluOpType.add,
                    replica_groups=[list(range(N_CORES))],
                    ins=[cc_in.opt()],
                    outs=[cc_out.opt()],
                )
                nc.gpsimd.dma_start(out_p[:], cc_out[:])

            wop_cm.__exit__(None, None, None)

    nc.finalize()
    return nc


def _execute(inputs: dict, trace: bool = False):
    from concourse.bass_utils import run_bass_kernel_spmd

    start_pos = int(np.asarray(inputs["start_pos"]))
    assert start_pos + 1 == MAX_SEQ, f"kernel hardcoded for klen=4096, got {start_pos}"

    in_maps = prepare_inputs(
        inputs["x"],
        inputs["freqs_cis"],
        inputs["cache_k"],
        inputs["cache_v"],
        inputs["wq"],
        inputs["bq"],
        inputs["wk"],
        inputs["bk"],
        inputs["wv"],
        inputs["bv"],
        inputs["wo"],
        inputs["bo"],
    )
    nc = build_graph()
    import os

    kw = {}
    if trace and os.environ.get("TRACE_ALL") == "1":
        kw["trace_cores"] = list(range(N_CORES))
    res = run_bass_kernel_spmd(
        nc, in_maps, core_ids=list(range(N_CORES)), trace=trace, **kw
    )
    # unshard: core i holds batch-row i of the output
    out = np.stack(
        [np.asarray(res.results[i]["out"]).reshape(DIM) for i in range(N_CORES)]
    ).reshape(NB, 1, DIM).astype(np.float32)
    return out, res


def kernel(**inputs) -> np.ndarray:
    return _execute(inputs, trace=False)[0]



# revision 57
# speedup vs baseline: 1.0311x; 1.0311x over previous
"""Trainium2 Bass kernel for GQA decode attention (nn_Attention_37890201485423).

Tensor-parallel over KV heads: 8 cores x (1 KV head + 5 query heads each).
o_proj is row-sharded; the cross-core reduction is a single ReduceScatter
(input [8 batches, 5120] row-major -> core i ends with batch i's summed
output row; the host stacks the 8 rows, a pure unshard).

Layout strategy (all host-side prep is numpy; all module math runs on device):
  - All large arrays ship to HBM as bf16 (the matmuls consume bf16 operands;
    the f32 baseline cast during the DMA anyway). This halves HBM traffic,
    the binding resource: ~32.5 MB/core at ~330 GB/s effective = ~100 us.
  - K cache shard is shipped pre-transposed per batch as [d=128, t=4096] so
    the scores matmul uses K^T chunks as the stationary operand directly.
  - V cache shard is shipped as 128-token chunks with a ones column appended
    ([t, 129]); the AV matmul then produces numerator and softmax denominator
    in one accumulation group.
  - RoPE runs on device as one matmul with a 128x128 block-diagonal rotation
    matrix built from freqs_cis (f32 for precision; everything else bf16 with
    fp32 PSUM accumulation). Measured rel err ~6e-3 (gate 2e-2).

Schedule (the DMA stream is the critical resource; everything chases it):
  - One ordered SWDGE stream: wqkv -> whole per-batch K/V transfers (8KB
    lines; 6-deep tile prefetch so buffer recycling never stalls the stream
    while attention catches up) -> wo in 10 chunks.
  - A completion barrier keys wo to K7's completion: order-only deps would
    let wo bytes interleave with late KV batches across the 16 queues,
    delaying V7 and serializing o_proj behind the whole stream; anchoring on
    K7 lets wo's descriptor-gen overlap only V7's transfer. The six o_proj
    rounds (4x1024 + 2x512 cols, PSUM double-buffered per size) chase the wo
    chunk arrivals and finish ~3 us after the last byte.
  - Software pipelining: scores(b+1) is emitted before AV(b) so the PE fills
    the ACT exp latency; attention starts ~1.5 batches behind the stream
    (projections+rope gate it) and catches up by mid-stream.
  - Heater matmuls (dep-free junk on the PE) pad the per-batch stop-start
    bursts into sustained activity; without them the HAM clock gate never
    sees a fully-busy 3.4us window and every matmul runs at 1.2 GHz instead
    of 2.4 (cold attention falls BEHIND the DMA stream).
  - o_proj bias is folded into the DVE PSUM->SBUF cast (tensor_add with a
    host-replicated [8, 5120] bias), removing all bias matmuls.
  - A tiny warmup ReduceScatter, pinned early into the gpsimd chain, absorbs
    the ncfw cold start (unpinned, the Tile scheduler parks it right before
    the real RS where it serializes with it).
  - Constants (identity, memsets) are issued at the head of the gpsimd queue:
    behind the big dma_starts they run 20+ us late (ring backpressure) and
    stall the first projection matmul.

Measured: ~147-162 us on 8 cores (median ~153; baseline was ~245; rare
skew outliers to ~225). Core 0's span includes cross-core launch skew
(5-30 us, run-to-run) absorbed at the final ReduceScatter, plus ~7 us RS
mesh stages and ~6 us end-of-program drain. The local pipeline (first
instruction -> RS trigger) is ~116-119 us against a ~108 us DMA-stream
floor (32.5 MB/core at ~330 GB/s effective + preamble).
"""

import sys

import numpy as np

if "/opt/trn_rl_repo" not in sys.path:
    sys.path.insert(0, "/opt/trn_rl_repo")


def _install_ntff_hook():
    """The container's antenv stub lacks axon_hooks; recreate it so
    run_bass_kernel_spmd(trace=True) can capture NTFF profiles via the
    libaxon ctypes path (mirrors trn_agent_boot.trn_boot)."""
    import types

    if "antenv.axon_hooks" in sys.modules:
        return
    mod = types.ModuleType("antenv.axon_hooks")
    mod._hook = None

    def set_axon_ntff_profile_hook(h):
        mod._hook = h

    def get_axon_ntff_profile_hook():
        return mod._hook

    mod.set_axon_ntff_profile_hook = set_axon_ntff_profile_hook
    mod.get_axon_ntff_profile_hook = get_axon_ntff_profile_hook
    sys.modules["antenv.axon_hooks"] = mod
    try:
        import antenv

        antenv.axon_hooks = mod
    except ImportError:
        pass
    try:
        boot_dir = "/root/.axon_site/trn_agent_boot"
        if boot_dir not in sys.path:
            sys.path.insert(0, boot_dir)
        import trn_boot

        hook = trn_boot._ntff_profile_via_ctypes("/opt/axon/libaxon_pjrt.so")
        if hook is not None:
            mod._hook = hook
    except Exception:
        pass


_install_ntff_hook()

DIM, N_HEADS, N_KV, HEAD_DIM = 5120, 40, 8, 128
MAX_BS, MAX_SEQ = 8, 4096
NB = 8  # batch
N_CORES = 8
N_REP = N_HEADS // N_KV  # 5 query heads per kv head
HPC = N_REP  # heads per core
QD = HPC * HEAD_DIM  # 640, per-core q/o width
WKV = QD + 2 * HEAD_DIM  # 896: concat q|k|v projection width per core
KD = 40  # contraction chunks for DIM
NCH = 32  # 4096 / 128 token chunks
VE = HEAD_DIM + 1  # 129, v chunk width with ones column
# o_proj rounds (out col base, width): 4x1024 + 2x512 cols, double-buffered
# in PSUM per size-tag, each round chasing its wo chunk's DMA arrival (wo is
# last in the DMA stream); the last round is small to shorten the tail chain.
OP_ROUNDS = [(0, 1024), (1024, 1024), (2048, 1024), (3072, 1024), (4096, 512), (4608, 512)]
SCALE = 1.0 / float(np.sqrt(HEAD_DIM))
WARMUP_RS = True


def _build_rope_matrix(freqs_cis: np.ndarray) -> np.ndarray:
    """lhsT for the rope matmul: out = lhsT.T @ rhs applies the rotation A.

    A[2i,2i]=cos_i, A[2i,2i+1]=-sin_i, A[2i+1,2i]=sin_i, A[2i+1,2i+1]=cos_i
    (matches reference _apply_rope with interleaved even/odd pairs).
    """
    cos = np.asarray(freqs_cis, np.float32)[0, :, 0]
    sin = np.asarray(freqs_cis, np.float32)[0, :, 1]
    A = np.zeros((HEAD_DIM, HEAD_DIM), np.float32)
    idx = np.arange(HEAD_DIM // 2)
    A[2 * idx, 2 * idx] = cos
    A[2 * idx, 2 * idx + 1] = -sin
    A[2 * idx + 1, 2 * idx] = sin
    A[2 * idx + 1, 2 * idx + 1] = cos
    return np.ascontiguousarray(A.T)


def _part_major(w: np.ndarray) -> np.ndarray:
    """[K*128, N] -> [128, K*N] with chunk k in columns k*N:(k+1)*N."""
    k = w.shape[0] // 128
    return np.ascontiguousarray(
        w.reshape(k, 128, w.shape[1]).transpose(1, 0, 2).reshape(128, -1)
    )


def prepare_inputs(x, freqs_cis, cache_k, cache_v, wq, bq, wk, bk, wv, bv, wo, bo):
    """Returns per-core in_maps (list of dicts of numpy arrays).

    All large arrays are shipped to HBM as bf16 (the matmuls consume bf16
    operands anyway — the f32 baseline cast during the DMA; pre-casting on
    the host halves HBM traffic, which is the bottleneck). arope stays f32
    so the rope rotation keeps f32 precision.
    """
    import ml_dtypes

    bf16 = ml_dtypes.bfloat16
    x = np.asarray(x, np.float32).reshape(NB, DIM)
    arope = _build_rope_matrix(freqs_cis)

    # xt: stationary operand layout for the projections.
    # xt[:, k*8:(k+1)*8][p, c] = x[c, k*128+p]
    xs = x.reshape(NB, KD, 128)
    xt = np.ascontiguousarray(xs.transpose(2, 1, 0).reshape(128, KD * NB))

    # replicated across the 8 batch rows so the DVE can add it during the
    # o_proj PSUM->SBUF cast (kills the 10 bias matmuls on the PE)
    bo8 = np.tile((np.asarray(bo, np.float32) / N_CORES).reshape(1, DIM), (NB, 1))
    wq, wk, wv = (np.asarray(a, np.float32) for a in (wq, wk, wv))
    bqf = np.asarray(bq, np.float32).reshape(N_HEADS * HEAD_DIM)
    bkf = np.asarray(bk, np.float32).reshape(N_KV * HEAD_DIM)
    bvf = np.asarray(bv, np.float32).reshape(N_KV * HEAD_DIM)

    in_maps = []
    for i in range(N_CORES):
        # concat q|k|v slices: [5120, 896]
        w_cat = np.concatenate(
            [
                wq[:, i * QD : (i + 1) * QD],
                wk[:, i * HEAD_DIM : (i + 1) * HEAD_DIM],
                wv[:, i * HEAD_DIM : (i + 1) * HEAD_DIM],
            ],
            axis=1,
        )
        b_cat = np.concatenate(
            [
                bqf[i * QD : (i + 1) * QD],
                bkf[i * HEAD_DIM : (i + 1) * HEAD_DIM],
                bvf[i * HEAD_DIM : (i + 1) * HEAD_DIM],
            ]
        ).reshape(1, WKV)
        # wo: partition-major, then column-round-major so each o_proj round's
        # block is one contiguous DMA chunk
        wo_pm3 = _part_major(
            np.asarray(wo, np.float32)[i * QD : (i + 1) * QD, :]
        ).reshape(128, HPC, DIM)
        wo_i = np.ascontiguousarray(
            np.concatenate(
                [
                    wo_pm3[:, :, cb : cb + cw].reshape(128, HPC * cw)
                    for cb, cw in OP_ROUNDS
                ],
                axis=1,
            )
        )
        kt_i = np.asarray(cache_k, np.float32)[:, :, i, :].transpose(
            0, 2, 1
        )  # [8, 128, 4096]
        v_raw = np.asarray(cache_v, np.float32)[:, :, i, :].reshape(
            NB, NCH, 128, HEAD_DIM
        )
        v_ext = np.ones((NB, NCH, 128, VE), np.float32)
        v_ext[..., :HEAD_DIM] = v_raw
        v_i = v_ext.transpose(0, 2, 1, 3).reshape(NB, 128, NCH * VE)  # [8,128,4128]
        kv_i = np.ascontiguousarray(
            np.concatenate([kt_i, v_i], axis=2)
        )  # [8, 128, 8224]
        in_maps.append(
            dict(
                xt=xt.astype(bf16),
                wqkv=_part_major(w_cat).astype(bf16),
                bqkv=np.ascontiguousarray(b_cat).astype(bf16),
                arope=arope,
                kv=kv_i.astype(bf16),
                wo=wo_i.astype(bf16),
                bo8=bo8.astype(bf16),
            )
        )
    return in_maps


def build_graph():
    import concourse.mybir as mybir
    from concourse import bacc
    from concourse.masks import make_identity
    from concourse.tile import TileContext

    from concourse.bass import _add_dep_helper

    f32 = mybir.dt.float32
    bf16 = mybir.dt.bfloat16
    nc = bacc.Bacc(num_devices=N_CORES, name="attn_decode_tp8")

    # Pin the big-DMA stream order (the Tile scheduler otherwise reorders the
    # gpsimd queue and can push weight loads behind the KV cache stream).
    _prev_dma = [None]

    def ordered(bi):
        if _prev_dma[0] is not None:
            _add_dep_helper(
                bi.ins, _prev_dma[0].ins, sync=False, reason="dma stream order"
            )
        _prev_dma[0] = bi
        return bi

    xt_p = nc.declare_dram_parameter("xt", [128, KD * NB], bf16, isOutput=False)
    wqkv_p = nc.declare_dram_parameter("wqkv", [128, KD * WKV], bf16, isOutput=False)
    bqkv_p = nc.declare_dram_parameter("bqkv", [1, WKV], bf16, isOutput=False)
    arope_p = nc.declare_dram_parameter("arope", [128, 128], f32, isOutput=False)
    kv_p = nc.declare_dram_parameter(
        "kv", [NB, 128, MAX_SEQ + NCH * VE], bf16, isOutput=False
    )
    wo_p = nc.declare_dram_parameter("wo", [128, HPC * DIM], bf16, isOutput=False)
    bo8_p = nc.declare_dram_parameter("bo8", [NB, DIM], bf16, isOutput=False)
    # per-core output: batch-row core_id of the full [NB, DIM] result
    # (ReduceScatter leaves core i with row i; the host stacks the rows)
    out_p = nc.declare_dram_parameter("out", [1, DIM], f32, isOutput=True)

    Exp = mybir.ActivationFunctionType.Exp

    with TileContext(nc, num_cores=N_CORES) as tc:
        with (
            tc.tile_pool(name="const", bufs=1) as constp,
            tc.tile_pool(name="persist", bufs=1) as pers,
            tc.tile_pool(name="dram", bufs=1, space="DRAM") as dramp,
            tc.tile_pool(name="ktp", bufs=3) as ktp,
        ):
            identity = constp.tile([128, 128], f32)
            ones8 = constp.tile([1, NB], bf16)
            bo8_sb = constp.tile([NB, DIM], bf16)
            arope_sb = constp.tile([128, 128], f32)
            # const compute at the very head of the gpsimd queue: the big
            # dma_starts below backpressure the gpsimd engine on the DMA ring,
            # so anything queued after them runs 20+ us late — and the first
            # projection matmul needs ones8
            make_identity(nc, identity)
            nc.gpsimd.memset(ones8[:], 1.0)
            nc.sync.dma_start(arope_sb[:], arope_p[:])
            bqkv_sb = constp.tile([1, WKV], bf16)
            ordered(nc.gpsimd.dma_start(bqkv_sb[:], bqkv_p[:]))

            # tiny warm-up collective: absorbs the ncfw cold-start cost so the
            # real output ReduceScatter at the tail runs at steady-state latency
            warm_in = dramp.tile([1, NB * N_CORES], bf16)
            warm_out = dramp.tile([1, NB], bf16)
            nc.sync.dma_start(warm_in[:], bqkv_p[:, 0 : NB * N_CORES])
            # pin the warmup trigger into the ordered gpsimd chain — left to
            # the Tile scheduler it lands right before the real ReduceScatter
            # and serializes with it instead of warming the ncfw path early
            if WARMUP_RS:
                ordered(
                    nc.gpsimd.collective_compute(
                        "ReduceScatter",
                        mybir.AluOpType.add,
                        replica_groups=[list(range(N_CORES))],
                        ins=[warm_in.opt()],
                        outs=[warm_out.opt()],
                    )
                )

            qr_sb = pers.tile([128, NB * HPC], bf16)  # roped q^T, cols b*5+h
            knT_sb = pers.tile([128, NB], bf16)  # roped new-k^T, cols b
            xv_sb = pers.tile([NB, HEAD_DIM], bf16)  # new v rows
            # flattened copy on partition 0 (matmul operands need base
            # partition 0): cols b*VE..b*VE+127 = v_new[b], col b*VE+128 = 1.0
            xvf_sb = pers.tile([1, NB * VE], bf16)
            nc.gpsimd.memset(xvf_sb[:], 1.0)
            attnT_sb = pers.tile([128, HPC * NB], bf16)  # cols h*8+b

            kt_tiles, v_tiles = {}, {}
            kv_last = [None]  # last kv DMA instruction
            kv_barrier = [None]  # V7 first half: wo gen overlaps only the last ~1MB
            KVW = MAX_SEQ + NCH * VE  # 8224
            HCH = NCH // 2  # chunks per exp half

            def load_kv(b):
                # four DMAs per batch (K and V in halves, ~0.5 MB each):
                # fine-grained arrivals keep PE idle gaps well under the
                # ~3.4us HAM re-throttle window (the real compute chasing
                # each half is the heartbeat that keeps the PE at 2.4 GHz),
                # and region-level dep tracking lets scores/AV halves start
                # as soon as their half landed
                src = kv_p[b : b + 1].rearrange("b p w -> p (b w)")
                kt = ktp.tile([128, MAX_SEQ], bf16, tag="kt", bufs=6, name=f"kt{b}")
                vt = ktp.tile([128, NCH * VE], bf16, tag="vt", bufs=6, name=f"vt{b}")
                # whole-K / whole-V transfers (8KB lines, fewer ring handoffs
                # than the half-split variant); heaters now own HAM warmth so
                # arrival granularity only matters at the tail
                kv_barrier[0] = ordered(
                    nc.gpsimd.dma_start(kt[:], src[:, 0:MAX_SEQ])
                )
                kv_last[0] = ordered(
                    nc.gpsimd.dma_start(vt[:], src[:, MAX_SEQ:KVW])
                )
                kt_tiles[b] = kt[:]
                v_tiles[b] = vt[:]

            # ---------------- Phase A: projections + rope ----------------
            with (
                tc.tile_pool(name="pha", bufs=1) as pha,
                tc.tile_pool(name="wqp", bufs=2) as wqp,
                tc.tile_pool(name="phaps", bufs=1, space="PSUM") as phaps,
            ):
                xt_sb = pha.tile([128, KD * NB], bf16)
                ordered(nc.gpsimd.dma_start(xt_sb[:], xt_p[:]))

                GK = 20  # k-chunks per wqkv tile
                wq_tiles = []
                wq_last = None
                for g in range(KD // GK):
                    wq_sb = wqp.tile(
                        [128, GK * WKV], bf16, tag="wq", name=f"wq{g}"
                    )
                    wq_last = ordered(
                        nc.gpsimd.dma_start(
                            wq_sb[:], wqkv_p[:, g * GK * WKV : (g + 1) * GK * WKV]
                        )
                    )
                    wq_tiles.append(wq_sb)
                # start the first KV cache loads right behind the weights;
                # 6 buffers of prefetch depth so buffer recycling (kt(b+6)
                # waits scores(b)) never stalls the stream while attention is
                # still catching up
                for _pb in range(6):
                    load_kv(_pb)
                # bo8 is only needed in Phase C — keep it off the queue head
                ordered(nc.gpsimd.dma_start(bo8_sb[:], bo8_p[:]))

                qkv_ps = phaps.tile([NB, WKV], f32)
                nc.tensor.matmul(
                    qkv_ps[:, 0:512],
                    ones8[:],
                    bqkv_sb[:, 0:512],
                    start=True,
                    stop=False,
                )
                nc.tensor.matmul(
                    qkv_ps[:, 512:WKV],
                    ones8[:],
                    bqkv_sb[:, 512:WKV],
                    start=True,
                    stop=False,
                )
                for g in range(KD // GK):
                    wq_sb = wq_tiles[g]
                    for o in range(GK):
                        k = g * GK + o
                        lhsT = xt_sb[:, k * NB : (k + 1) * NB]
                        last = k == KD - 1
                        nc.tensor.matmul(
                            qkv_ps[:, 0:512],
                            lhsT,
                            wq_sb[:, o * WKV : o * WKV + 512],
                            start=False,
                            stop=last,
                        )
                        nc.tensor.matmul(
                            qkv_ps[:, 512:WKV],
                            lhsT,
                            wq_sb[:, o * WKV + 512 : (o + 1) * WKV],
                            start=False,
                            stop=last,
                        )

                # copy q|k parts to f32 for transposes; v part to bf16
                qk_sb = pha.tile([NB, QD + HEAD_DIM], f32)
                nc.vector.tensor_copy(qk_sb[:], qkv_ps[:, 0 : QD + HEAD_DIM])
                nc.vector.tensor_copy(xv_sb[:], qkv_ps[:, QD + HEAD_DIM : WKV])
                nc.sync.dma_start(
                    xvf_sb.rearrange("p (b e) -> p b e", e=VE)[:, :, 0:HEAD_DIM],
                    xv_sb[:],
                )

                qkT_sb = pha.tile([128, NB * HPC + NB], f32)
                qkT_q3 = qkT_sb[:, 0 : NB * HPC].rearrange("p (b h) -> p b h", h=HPC)
                for h in range(HPC):
                    tq_ps = phaps.tile([128, NB], f32, tag="tq", bufs=2)
                    nc.tensor.transpose(
                        tq_ps[:],
                        qk_sb[:, h * HEAD_DIM : (h + 1) * HEAD_DIM],
                        identity[0:NB, 0:NB],
                    )
                    nc.vector.tensor_copy(qkT_q3[:, :, h], tq_ps[:])
                tk_ps = phaps.tile([128, NB], f32, tag="tq", bufs=2)
                nc.tensor.transpose(
                    tk_ps[:], qk_sb[:, QD : QD + HEAD_DIM], identity[0:NB, 0:NB]
                )
                nc.vector.tensor_copy(qkT_sb[:, NB * HPC : NB * HPC + NB], tk_ps[:])

                qkr_ps = phaps.tile([128, NB * HPC + NB], f32)
                nc.tensor.matmul(
                    qkr_ps[:], arope_sb[:], qkT_sb[:], start=True, stop=True
                )
                nc.vector.tensor_copy(qr_sb[:], qkr_ps[:, 0 : NB * HPC])
                nc.vector.tensor_copy(
                    knT_sb[:], qkr_ps[:, NB * HPC : NB * HPC + NB]
                )

            # ---------------- Phase B: attention per batch ----------------
            # Straight per-batch order (scores -> exp -> AV -> normalize ->
            # transpose); per-batch K/V arrivals keep the PE chasing the DMA
            # stream with only sub-us ACT/DVE ping-pong stalls.
            wo_tiles = []
            wop_cm = tc.tile_pool(name="wop", bufs=1)
            wop = wop_cm.__enter__()
            with (
                tc.tile_pool(name="psm", bufs=2) as psm,
                tc.tile_pool(name="aps", bufs=2, space="PSUM") as aps,
            ):
                attnT_3 = attnT_sb.rearrange("p (h b) -> p h b", b=NB)

                s_tiles, sn_tiles = {}, {}

                def scores(b):
                    ktb = kt_tiles[b]
                    s_ps = aps.tile([128, NCH * HPC], f32, tag="s", name=f"s{b}")
                    for c in range(NCH):
                        # last chunk: only 127 rows — cache position 4095 is
                        # stale (the new token is handled separately below)
                        w = 127 if c == NCH - 1 else 128
                        nc.tensor.matmul(
                            s_ps[0:w, c * HPC : (c + 1) * HPC],
                            ktb[:, c * 128 : c * 128 + w],
                            qr_sb[:, b * HPC : (b + 1) * HPC],
                            start=True,
                            stop=True,
                        )
                    # new token's score row: [1, 5] from its roped k column
                    sn_ps = aps.tile([1, HPC], f32, tag="sn", name=f"sn{b}")
                    nc.tensor.matmul(
                        sn_ps[:],
                        knT_sb[:, b : b + 1],
                        qr_sb[:, b * HPC : (b + 1) * HPC],
                        start=True,
                        stop=True,
                    )
                    s_tiles[b], sn_tiles[b] = s_ps, sn_ps

                scores(0)
                for b in range(NB):
                    if b + 6 < NB:
                        load_kv(b + 6)
                    # software pipeline: scores(b+1) is emitted before
                    # AV(b) so the PE fills the ACT exp(b) latency and the
                    # per-batch serial PE->ACT->PE chain overlaps across
                    # batches (catch-up rate ~4.3us/batch instead of ~6)
                    if b + 1 < NB:
                        scores(b + 1)
                    # ---- AV + normalize for batch b ----
                    s_ps, sn_ps = s_tiles[b], sn_tiles[b]
                    vb = v_tiles[b]
                    p_sb = psm.tile([128, NCH * HPC], bf16, tag="p")
                    pn_sb = psm.tile([1, HPC], bf16, tag="pn")
                    for hf in range(2):
                        sl = slice(hf * HCH * HPC, (hf + 1) * HCH * HPC)
                        nc.scalar.activation(
                            p_sb[:, sl], s_ps[:, sl], Exp, scale=SCALE
                        )
                    nc.scalar.activation(pn_sb[:], sn_ps[:], Exp, scale=SCALE)
                    o_ps = aps.tile([HPC, VE], f32, tag="o")
                    for c in range(NCH):
                        w = 127 if c == NCH - 1 else 128
                        nc.tensor.matmul(
                            o_ps[:],
                            p_sb[0:w, c * HPC : (c + 1) * HPC],
                            vb[0:w, c * VE : (c + 1) * VE],
                            start=(c == 0),
                            stop=False,
                        )
                    # new token's AV contribution (k=1 contraction)
                    nc.tensor.matmul(
                        o_ps[:],
                        pn_sb[:],
                        xvf_sb[:, b * VE : (b + 1) * VE],
                        start=False,
                        stop=True,
                    )
                    r_sb = psm.tile([HPC, 1], f32, tag="r")
                    nc.vector.reciprocal(r_sb[:], o_ps[:, HEAD_DIM : HEAD_DIM + 1])
                    attn_b = psm.tile([HPC, HEAD_DIM], f32, tag="attn_b")
                    nc.vector.tensor_scalar_mul(
                        attn_b[:], o_ps[:, 0:HEAD_DIM], r_sb[:]
                    )
                    ta_ps = aps.tile([128, HPC], f32, tag="ta", bufs=1)
                    nc.tensor.transpose(
                        ta_ps[:], attn_b[:], identity[0:HPC, 0:HPC]
                    )
                    nc.vector.tensor_copy(attnT_3[:, :, b], ta_ps[:])
                    # heater matmuls: fill the per-batch DMA-wait gap with
                    # dep-free junk work so the PE's HAM activity window stays
                    # saturated — otherwise the stop-start attention bursts
                    # never sustain the ~3.4us needed to hold the 2.4 GHz
                    # clock and every real matmul runs at half speed
                    # Heater budget: early batches (attention catching up to
                    # the stream after the projections+rope head start) get a
                    # full fill; late batches are data-gated with real idle,
                    # so a few heaters keep HAM warm into the o_proj rounds;
                    # batch 7 gets none (it's on the tail critical path).
                    nheat = 4 if b < 4 else 3
                    if nheat:
                        heat_ps = aps.tile([128, 128], f32, tag="heat", bufs=1)
                        for _ in range(nheat):
                            nc.tensor.matmul(
                                heat_ps[:], arope_sb[:], arope_sb[:],
                                start=True, stop=True,
                            )

                # wo stream: strictly after the last KV byte (a completion
                # barrier — order-only deps would let wo bytes interleave
                # with late KV batches and delay V7, serializing o_proj
                # behind the whole stream), in 10 chunks on the
                # gpsimd chain so the o_proj rounds chase the arrivals
                wo_sb = wop.tile([128, HPC * DIM], bf16)
                W = HPC * DIM
                NWC = 10
                for q in range(NWC):
                    bi = ordered(
                        nc.gpsimd.dma_start(
                            wo_sb[:, q * W // NWC : (q + 1) * W // NWC],
                            wo_p[:, q * W // NWC : (q + 1) * W // NWC],
                        )
                    )
                    if q == 0:
                        # key the barrier to V7's FIRST half: wo descriptor
                        # gen + first bytes overlap only the last ~1MB of KV,
                        # cutting the post-stream bubble without starving the
                        # attention tail
                        _add_dep_helper(
                            bi.ins,
                            kv_barrier[0].ins,
                            sync=True,
                            reason="wo after kv stream (V7 1st half)",
                        )
                wo_tiles.append(wo_sb)

            # ---------------- Phase C: o_proj + single AllReduce ----------------
            # 5 rounds of 1024 out-cols, double-buffered in PSUM; each round's
            # DVE cast + staging DMA overlaps the next round's matmuls. One
            # AllReduce over the full [8, 5120] bf16 output at the end (one
            # ~8us mesh traversal instead of two serialized ones).
            with (
                tc.tile_pool(name="opsp", bufs=1, space="PSUM") as opsp,
                tc.tile_pool(name="oop", bufs=1) as oop,
            ):
                # single ReduceScatter at the end: the last collective's entry
                # is bounded by the slowest core's chain end no matter how the
                # RS is phased, so extra phases just append their ~7us mesh.
                cc_in = dramp.tile([NB, DIM], bf16, tag="cci")
                cc_out = dramp.tile([1, DIM], bf16, tag="cco")

                for r, (cb, cw) in enumerate(OP_ROUNDS):
                    blk = HPC * cb  # wo tile base of this round's block
                    op_ps = opsp.tile(
                        [NB, cw], f32, tag=f"op{cw}", bufs=2, name=f"op{r}"
                    )
                    for h in range(HPC):
                        for j in range(cw // 512):
                            nc.tensor.matmul(
                                op_ps[:, j * 512 : (j + 1) * 512],
                                attnT_sb[:, h * NB : (h + 1) * NB],
                                wo_tiles[0][
                                    :,
                                    blk + h * cw + j * 512 : blk + h * cw + (j + 1) * 512,
                                ],
                                start=(h == 0),
                                stop=(h == HPC - 1),
                            )
                    oo_sb = oop.tile(
                        [NB, cw], bf16, tag=f"oo{cw}", bufs=2, name=f"oo{r}"
                    )
                    # cast + bias in one DVE pass
                    nc.vector.tensor_add(
                        oo_sb[:], op_ps[:], bo8_sb[:, cb : cb + cw]
                    )
                    nc.sync.dma_start(cc_in[:, cb : cb + cw], oo_sb[:])
                # ReduceScatter semantics: input [NB, DIM] row-major -> chunk
                # i = batch-row i, so core i ends up with batch i's summed
                # output; the host stacks the rows (pure unshard). Half the
                # wire of an AllReduce.
                nc.gpsimd.collective_compute(
                    "ReduceScatter",
                    mybir.AluOpType.add,
                    replica_groups=[list(range(N_CORES))],
                    ins=[cc_in.opt()],
                    outs=[cc_out.opt()],
                )
                nc.gpsimd.dma_start(out_p[:], cc_out[:])

            wop_cm.__exit__(None, None, None)

    nc.finalize()
    return nc


def _execute(inputs: dict, trace: bool = False):
    from concourse.bass_utils import run_bass_kernel_spmd

    start_pos = int(np.asarray(inputs["start_pos"]))
    assert start_pos + 1 == MAX_SEQ, f"kernel hardcoded for klen=4096, got {start_pos}"

    in_maps = prepare_inputs(
        inputs["x"],
        inputs["freqs_cis"],
        inputs["cache_k"],
        inputs["cache_v"],
        inputs["wq"],
        inputs["bq"],
        inputs["wk"],
        inputs["bk"],
        inputs["wv"],
        inputs["bv"],
        inputs["wo"],
        inputs["bo"],
    )
    nc = build_graph()
    import os

    kw = {}
    if trace and os.environ.get("TRACE_ALL") == "1":
        kw["trace_cores"] = list(range(N_CORES))
    res = run_bass_kernel_spmd(
        nc, in_maps, core_ids=list(range(N_CORES)), trace=trace, **kw
    )
    # unshard: core i holds batch-row i of the output
    out = np.stack(
        [np.asarray(res.results[i]["out"]).reshape(DIM) for i in range(N_CORES)]
    ).reshape(NB, 1, DIM).astype(np.float32)
    return out, res


def kernel(**inputs) -> np.ndarray:
    return _execute(inputs, trace=False)[0]

